# revision 25
# baseline (speedup 1.0000x reference)
"""MoE layer (router + top-k dispatch + per-expert FFN + weighted combine)
on 8 Trainium2 NeuronCores.

Sharding strategy (expert-parallel, host-side dispatch):
  - Core e owns expert e's weights (W1[e], W2[e], b1[e], b2[e]).
  - The host computes the top-k routing to decide WHICH tokens go to which
    core (the dispatch step of the sharding), gathers each expert's tokens,
    and ships them transposed ([D, C] token-minor) so both FFN GEMMs run
    with contraction on the partition axis and zero on-device transposes.
  - Each core re-computes the router (x @ Wg -> softmax) on-device for its
    own tokens to obtain the combine weight probs[token, e]; for a token
    routed to expert e that probability IS the reference combine weight.
  - Device output is w * (relu(x @ W1 + b1) @ W2 + b2), transposed [O, C].
  - The host unshard step scatters-adds each expert's token columns back
    into the [B, O] output (token indices are unique within one expert).

Compute is bf16 (fp32 PSUM accumulation); combine weights stay fp32.
"""

import numpy as np
import ml_dtypes
import bass_rust

import concourse.bass as bass
import concourse.mybir as mybir
import concourse.tile as tile
from concourse.bass_utils import run_bass_kernel_spmd

P = 128
N_CORES = 8
CHUNK = 512

def _normalize_sync_waits(nc):
    """The walrus build in this toolchain rejects >1 sync wait on a single
    instruction (setupSyncWait: "Too many sync wait commands"), while Tile's
    semaphore assignment freely emits several. Hoist all but one wait of each
    instruction onto same-engine NOPs placed immediately before it — the
    engine stream is in-order, so stalling at the NOPs is semantically
    identical to a multi-wait instruction."""
    count = 0
    for f in nc.m.functions:
        for bb in f.blocks:
            out = []
            changed = False
            for ins in bb.instructions:
                si = ins.sync_info
                if si is not None and len(si.on_wait) > 1:
                    waits = list(si.on_wait)
                    for w in waits[:-1]:
                        count += 1
                        out.append(
                            mybir.InstNoOp(
                                name=f"I-nw{count}",
                                ins=[],
                                outs=[],
                                engine=ins.engine,
                                sync_info=bass_rust.SyncInfo(
                                    on_wait=[w], on_update=[]
                                ),
                            )
                        )
                    ins.sync_info = bass_rust.SyncInfo(
                        on_wait=[waits[-1]], on_update=list(si.on_update)
                    )
                    changed = True
                out.append(ins)
            if changed:
                bb.instructions = out
    return nc


def _build_program(D, H, O, E, C, chunks):
    f32, bf16 = mybir.dt.float32, mybir.dt.bfloat16
    KD, MH, MO = D // P, H // P, O // P
    AF = mybir.ActivationFunctionType
    ALU = mybir.AluOpType

    nc = bass.Bass()
    xT = nc.declare_dram_parameter("xT", [D, C], bf16, isOutput=False)
    w1 = nc.declare_dram_parameter("w1", [D, H], bf16, isOutput=False)
    w2 = nc.declare_dram_parameter("w2", [H, O], bf16, isOutput=False)
    wg = nc.declare_dram_parameter("wg", [D, E], bf16, isOutput=False)
    b1p = nc.declare_dram_parameter("b1p", [P, MH], f32, isOutput=False)
    b2p = nc.declare_dram_parameter("b2p", [P, MO], f32, isOutput=False)
    sel = nc.declare_dram_parameter("sel", [P, 2], bf16, isOutput=False)
    yT = nc.declare_dram_parameter("yT", [O, C], f32, isOutput=True)

    with tile.TileContext(nc) as tc:
        with (
            tc.tile_pool(name="weights", bufs=1) as wpool,
            tc.tile_pool(name="dram", bufs=1, space="DRAM") as dram,
            tc.tile_pool(name="xc", bufs=2) as xcpool,
            tc.tile_pool(name="h", bufs=1) as hpool,
            tc.tile_pool(name="ex", bufs=2) as epool,
            tc.tile_pool(name="wch", bufs=2) as wpool2,
            tc.tile_pool(name="ob", bufs=4) as outpool,
            tc.tile_pool(name="ps_h", bufs=4, space="PSUM") as ps_h,
            tc.tile_pool(name="ps_y", bufs=2, space="PSUM") as ps_y,
            tc.tile_pool(name="ps_r", bufs=1, space="PSUM") as ps_r,
            tc.tile_pool(name="ps_w", bufs=1, space="PSUM") as ps_w,
        ):
            # Small latency-critical inputs first (program order sets DMA
            # priority): the router needs only wg + the first token chunk, so
            # PE work starts ~11us in instead of behind 16MB of weights. DMA
            # *trigger* issue costs ~0.6-1us of queue time each, so keep the
            # count low: 4 H-blocks for w1 (GEMM1 group mh only reads one
            # 128-col slice per kd tile, so the first 2MB block unblocks it),
            # 2 halves for w2 (not needed until GEMM2, ~60us in).
            wg_sb = wpool.tile([P, KD, E], bf16)
            b1_sb = wpool.tile([P, MH], f32)
            nc.gpsimd.dma_start(b1_sb[:], b1p[:])
            b2_sb = wpool.tile([P, MO], f32)
            nc.gpsimd.dma_start(b2_sb[:], b2p[:])
            sel_sb = wpool.tile([P, 2], bf16)
            nc.gpsimd.dma_start(sel_sb[:], sel[:])

            xT_r = xT.rearrange("(kd p) c -> p kd c", p=P)
            xc0 = xcpool.tile([P, KD, CHUNK], bf16, tag="xc")
            nc.sync.dma_start(xc0[:, :, : chunks[0]], xT_r[:, :, : chunks[0]])
            nc.sync.dma_start(wg_sb[:], wg.rearrange("(kd p) e -> p kd e", p=P))

            # First H-block as 8 small 2D DMAs: 2D triggers issue in ~0.7us
            # vs ~9us of descriptor generation for the merged 3D form, so
            # GEMM1's first groups unblock sooner. Later blocks merged — their
            # trigger time hides behind chunk-0 compute.
            w1_sb = wpool.tile([P, KD, H], bf16)
            w1_r = w1.rearrange("(kd p) h -> p kd h", p=P)
            HB = H // 4
            for kd in range(KD):
                nc.sync.dma_start(w1_sb[:, kd, :HB], w1_r[:, kd, :HB])
            for hb in range(1, 4):
                nc.sync.dma_start(
                    w1_sb[:, :, hb * HB : (hb + 1) * HB],
                    w1_r[:, :, hb * HB : (hb + 1) * HB],
                )
            w2_sb = wpool.tile([P, MH, O], bf16)
            w2_r = w2.rearrange("(kh p) o -> p kh o", p=P)
            for j in range(0, MH, MH // 2):
                nc.sync.dma_start(
                    w2_sb[:, j : j + MH // 2, :], w2_r[:, j : j + MH // 2, :]
                )

            wd_dram = dram.tile([2, C], f32)  # row 0: softmax denom, row 1: exp[e]

            c0 = 0
            for ci, N in enumerate(chunks):
                if ci == 0:
                    xc = xc0
                else:
                    xc = xcpool.tile([P, KD, CHUNK], bf16, tag="xc")
                    nc.gpsimd.dma_start(xc[:, :, :N], xT_r[:, :, c0 : c0 + N])

                # Router: logits^T = Wg^T @ x^T, exp, then one matmul against
                # [ones | onehot(e)] gives [denom; numer] per token.
                lg = ps_r.tile([E, CHUNK], f32, tag="lg")
                for kd in range(KD):
                    nc.tensor.matmul(
                        lg[:, :N],
                        wg_sb[:, kd, :],
                        xc[:, kd, :N],
                        start=(kd == 0),
                        stop=(kd == KD - 1),
                    )
                ex = epool.tile([P, CHUNK], bf16, tag="ex")
                nc.vector.memset(ex[:, :N], 0.0)
                nc.scalar.activation(ex[:E, :N], lg[:, :N], AF.Exp)
                wd_ps = ps_w.tile([2, CHUNK], f32, tag="wd")
                nc.tensor.matmul(wd_ps[:, :N], sel_sb[:], ex[:, :N], start=True, stop=True)
                wd_sb = epool.tile([2, CHUNK], f32, tag="wd_sb")
                nc.scalar.copy(wd_sb[:, :N], wd_ps[:, :N])
                nc.gpsimd.dma_start(wd_dram[:, c0 : c0 + N], wd_sb[:, :N])

                # Broadcast [1, N] -> [128, N] via a step-0 DRAM read, then
                # w = numer * (1 / denom).
                den_b = wpool2.tile([P, CHUNK], f32, tag="den")
                nc.gpsimd.dma_start(
                    den_b[:, :N], wd_dram[0:1, c0 : c0 + N].to_broadcast([P, N])
                )
                num_b = wpool2.tile([P, CHUNK], f32, tag="num")
                nc.gpsimd.dma_start(
                    num_b[:, :N], wd_dram[1:2, c0 : c0 + N].to_broadcast([P, N])
                )
                wch = wpool2.tile([P, CHUNK], f32, tag="wch")
                nc.vector.reciprocal(wch[:, :N], den_b[:, :N])
                nc.vector.tensor_mul(wch[:, :N], wch[:, :N], num_b[:, :N])

                # GEMM1: h^T = relu(W1^T @ x^T + b1), evicted to SBUF as bf16.
                hT = hpool.tile([P, MH, CHUNK], bf16, tag="h")
                for mh in range(MH):
                    ph = ps_h.tile([P, CHUNK], f32, tag="ph")
                    for kd in range(KD):
                        nc.tensor.matmul(
                            ph[:, :N],
                            w1_sb[:, kd, mh * P : (mh + 1) * P],
                            xc[:, kd, :N],
                            start=(kd == 0),
                            stop=(kd == KD - 1),
                        )
                    nc.scalar.activation(
                        hT[:, mh, :N], ph[:, :N], AF.Relu, bias=b1_sb[:, mh : mh + 1]
                    )

                # GEMM2: y^T = W2^T @ h^T, evicted as (y + b2) * w.
                for mo in range(MO):
                    py = ps_y.tile([P, CHUNK], f32, tag="py")
                    for kh in range(MH):
                        nc.tensor.matmul(
                            py[:, :N],
                            w2_sb[:, kh, mo * P : (mo + 1) * P],
                            hT[:, kh, :N],
                            start=(kh == 0),
                            stop=(kh == MH - 1),
                        )
                    ob = outpool.tile([P, CHUNK], f32, tag="ob")
                    nc.vector.scalar_tensor_tensor(
                        ob[:, :N],
                        py[:, :N],
                        b2_sb[:, mo : mo + 1],
                        wch[:, :N],
                        op0=ALU.add,
                        op1=ALU.mult,
                    )
                    nc.sync.dma_start(yT[mo * P : (mo + 1) * P, c0 : c0 + N], ob[:, :N])
                c0 += N
    return _normalize_sync_waits(nc)


def kernel(**inputs):
    x = np.ascontiguousarray(np.asarray(inputs["x"], dtype=np.float32))
    Wg = np.ascontiguousarray(np.asarray(inputs["Wg"], dtype=np.float32))
    W1 = np.asarray(inputs["W1"], dtype=np.float32)
    b1 = np.asarray(inputs["b1"], dtype=np.float32)
    W2 = np.asarray(inputs["W2"], dtype=np.float32)
    b2 = np.asarray(inputs["b2"], dtype=np.float32)
    k = int(np.asarray(inputs["k"]))

    B, D = x.shape
    E = Wg.shape[1]
    H = W1.shape[2]
    O = W2.shape[2]
    assert E == N_CORES, f"expert-per-core layout expects E == 8, got {E}"

    # Host-side dispatch: pick each token's top-k experts (softmax is
    # monotonic, so top-k on logits == top-k on probs).
    logits = x @ Wg
    kth = np.partition(logits, E - k, axis=1)[:, E - k]  # k-th largest per token
    routed = logits >= kth[:, None]  # [B, E] membership mask
    idx_per_e = [np.nonzero(routed[:, e])[0] for e in range(E)]
    counts = [len(ix) for ix in idx_per_e]

    # Capacity: pad the largest expert's token count to a multiple of 64.
    # Split into <=512-token chunks; keep every chunk >=256 (below that the
    # fixed per-matmul issue/LDWEIGHTS cost stops amortizing) by borrowing
    # from the previous full chunk.
    C = max(CHUNK, -(-max(counts) // 8) * 8)
    chunks = [CHUNK] * (C // CHUNK)
    rem = C % CHUNK
    if rem:
        if rem < 256 and chunks:
            chunks[-1] -= 256 - rem
            rem = 256
        chunks.append(rem)

    nc = _build_program(D, H, O, E, C, chunks)

    in_maps = []
    wg_bf = np.ascontiguousarray(Wg.astype(ml_dtypes.bfloat16))
    for e in range(E):
        idx = idx_per_e[e]
        pad = np.zeros(C, dtype=np.int64)
        pad[: counts[e]] = idx
        xT_e = np.ascontiguousarray(x[pad].T.astype(ml_dtypes.bfloat16))
        sel128 = np.zeros((P, 2), dtype=ml_dtypes.bfloat16)
        sel128[:E, 0] = 1.0
        sel128[e, 1] = 1.0
        in_maps.append(
            {
                "xT": xT_e,
                "w1": np.ascontiguousarray(W1[e].astype(ml_dtypes.bfloat16)),
                "w2": np.ascontiguousarray(W2[e].astype(ml_dtypes.bfloat16)),
                "wg": wg_bf,
                "b1p": np.ascontiguousarray(b1[e].reshape(H // P, P).T),
                "b2p": np.ascontiguousarray(b2[e].reshape(O // P, P).T),
                "sel": sel128,
            }
        )

    res = run_bass_kernel_spmd(nc, in_maps, core_ids=list(range(N_CORES)))
    globals()["_last_results"] = res

    out = np.zeros((B, O), dtype=np.float32)
    for e in range(E):
        cnt = counts[e]
        if cnt:
            yT_e = res.results[e]["yT"]
            out[idx_per_e[e]] += yT_e[:, :cnt].T
    return out


# revision 26
# speedup vs baseline: 1.0021x; 1.0021x over previous
"""MoE layer (router + top-k dispatch + per-expert FFN + weighted combine)
on 8 Trainium2 NeuronCores.

Sharding strategy (expert-parallel, host-side dispatch):
  - Core e owns expert e's weights (W1[e], W2[e], b1[e], b2[e]).
  - The host computes the top-k routing to decide WHICH tokens go to which
    core (the dispatch step of the sharding), gathers each expert's tokens,
    and ships them transposed ([D, C] token-minor) so both FFN GEMMs run
    with contraction on the partition axis and zero on-device transposes.
  - Each core re-computes the router (x @ Wg -> softmax) on-device for its
    own tokens to obtain the combine weight probs[token, e]; for a token
    routed to expert e that probability IS the reference combine weight.
  - Device output is w * (relu(x @ W1 + b1) @ W2 + b2), transposed [O, C].
  - The host unshard step scatters-adds each expert's token columns back
    into the [B, O] output (token indices are unique within one expert).

Compute is bf16 (fp32 PSUM accumulation); combine weights stay fp32.
"""

import numpy as np
import ml_dtypes
import bass_rust

import concourse.bass as bass
import concourse.mybir as mybir
import concourse.tile as tile
from concourse.bass_utils import run_bass_kernel_spmd

P = 128
N_CORES = 8
CHUNK = 512

def _normalize_sync_waits(nc):
    """The walrus build in this toolchain rejects >1 sync wait on a single
    instruction (setupSyncWait: "Too many sync wait commands"), while Tile's
    semaphore assignment freely emits several. Hoist all but one wait of each
    instruction onto same-engine NOPs placed immediately before it — the
    engine stream is in-order, so stalling at the NOPs is semantically
    identical to a multi-wait instruction."""
    count = 0
    for f in nc.m.functions:
        for bb in f.blocks:
            out = []
            changed = False
            for ins in bb.instructions:
                si = ins.sync_info
                if si is not None and len(si.on_wait) > 1:
                    waits = list(si.on_wait)
                    for w in waits[:-1]:
                        count += 1
                        out.append(
                            mybir.InstNoOp(
                                name=f"I-nw{count}",
                                ins=[],
                                outs=[],
                                engine=ins.engine,
                                sync_info=bass_rust.SyncInfo(
                                    on_wait=[w], on_update=[]
                                ),
                            )
                        )
                    ins.sync_info = bass_rust.SyncInfo(
                        on_wait=[waits[-1]], on_update=list(si.on_update)
                    )
                    changed = True
                out.append(ins)
            if changed:
                bb.instructions = out
    return nc


def _build_program(D, H, O, E, C, chunks):
    f32, bf16 = mybir.dt.float32, mybir.dt.bfloat16
    KD, MH, MO = D // P, H // P, O // P
    AF = mybir.ActivationFunctionType
    ALU = mybir.AluOpType

    nc = bass.Bass()
    xT = nc.declare_dram_parameter("xT", [D, C], bf16, isOutput=False)
    w1 = nc.declare_dram_parameter("w1", [D, H], bf16, isOutput=False)
    w2 = nc.declare_dram_parameter("w2", [H, O], bf16, isOutput=False)
    wg = nc.declare_dram_parameter("wg", [D, E], bf16, isOutput=False)
    b1p = nc.declare_dram_parameter("b1p", [P, MH], f32, isOutput=False)
    b2p = nc.declare_dram_parameter("b2p", [P, MO], f32, isOutput=False)
    sel = nc.declare_dram_parameter("sel", [P, 2], bf16, isOutput=False)
    yT = nc.declare_dram_parameter("yT", [O, C], f32, isOutput=True)

    with tile.TileContext(nc) as tc:
        with (
            tc.tile_pool(name="weights", bufs=1) as wpool,
            tc.tile_pool(name="dram", bufs=1, space="DRAM") as dram,
            tc.tile_pool(name="xc", bufs=2) as xcpool,
            tc.tile_pool(name="h", bufs=1) as hpool,
            tc.tile_pool(name="ex", bufs=2) as epool,
            tc.tile_pool(name="wch", bufs=2) as wpool2,
            tc.tile_pool(name="ob", bufs=4) as outpool,
            tc.tile_pool(name="ps_h", bufs=4, space="PSUM") as ps_h,
            tc.tile_pool(name="ps_y", bufs=2, space="PSUM") as ps_y,
            tc.tile_pool(name="ps_r", bufs=1, space="PSUM") as ps_r,
            tc.tile_pool(name="ps_w", bufs=1, space="PSUM") as ps_w,
        ):
            # Small latency-critical inputs first (program order sets DMA
            # priority): the router needs only wg + the first token chunk, so
            # PE work starts ~11us in instead of behind 16MB of weights. DMA
            # *trigger* issue costs ~0.6-1us of queue time each, so keep the
            # count low: 4 H-blocks for w1 (GEMM1 group mh only reads one
            # 128-col slice per kd tile, so the first 2MB block unblocks it),
            # 2 halves for w2 (not needed until GEMM2, ~60us in).
            wg_sb = wpool.tile([P, KD, E], bf16)
            b1_sb = wpool.tile([P, MH], f32)
            nc.gpsimd.dma_start(b1_sb[:], b1p[:])
            b2_sb = wpool.tile([P, MO], f32)
            nc.gpsimd.dma_start(b2_sb[:], b2p[:])
            sel_sb = wpool.tile([P, 2], bf16)
            nc.gpsimd.dma_start(sel_sb[:], sel[:])

            xT_r = xT.rearrange("(kd p) c -> p kd c", p=P)
            xc0 = xcpool.tile([P, KD, CHUNK], bf16, tag="xc")
            nc.sync.dma_start(xc0[:, :, : chunks[0]], xT_r[:, :, : chunks[0]])
            nc.sync.dma_start(wg_sb[:], wg.rearrange("(kd p) e -> p kd e", p=P))

            # First H-block as 8 small 2D DMAs: 2D triggers issue in ~0.7us
            # vs ~9us of descriptor generation for the merged 3D form, so
            # GEMM1's first groups unblock sooner. Later blocks merged — their
            # trigger time hides behind chunk-0 compute.
            w1_sb = wpool.tile([P, KD, H], bf16)
            w1_r = w1.rearrange("(kd p) h -> p kd h", p=P)
            HB = H // 4
            for kd in range(KD):
                nc.sync.dma_start(w1_sb[:, kd, :HB], w1_r[:, kd, :HB])
            for hb in range(1, 4):
                nc.sync.dma_start(
                    w1_sb[:, :, hb * HB : (hb + 1) * HB],
                    w1_r[:, :, hb * HB : (hb + 1) * HB],
                )
            w2_sb = wpool.tile([P, MH, O], bf16)
            w2_r = w2.rearrange("(kh p) o -> p kh o", p=P)
            for j in range(0, MH, MH // 2):
                nc.sync.dma_start(
                    w2_sb[:, j : j + MH // 2, :], w2_r[:, j : j + MH // 2, :]
                )

            wd_dram = dram.tile([2, C], f32)  # row 0: softmax denom, row 1: exp[e]

            # PE pre-warm: tiny matmuls on a resident constant fill the
            # otherwise-idle window before the first token chunk lands, so
            # the HAM clock gate flips to 2.4GHz before real matmuls start
            # (first ~3.4us of PE work otherwise runs at 1.2GHz).
            dummy = wpool.tile([P, 1], f32)
            nc.vector.memset(dummy[:], 1.0)
            lg_warm = ps_r.tile([E, CHUNK], f32, tag="lg")
            for _ in range(48):
                nc.tensor.matmul(
                    lg_warm[0:1, 0:1], dummy[:], dummy[:], start=True, stop=True
                )

            c0 = 0
            for ci, N in enumerate(chunks):
                if ci == 0:
                    xc = xc0
                else:
                    xc = xcpool.tile([P, KD, CHUNK], bf16, tag="xc")
                    nc.gpsimd.dma_start(xc[:, :, :N], xT_r[:, :, c0 : c0 + N])

                # Router: logits^T = Wg^T @ x^T, exp, then one matmul against
                # [ones | onehot(e)] gives [denom; numer] per token.
                lg = ps_r.tile([E, CHUNK], f32, tag="lg")
                for kd in range(KD):
                    nc.tensor.matmul(
                        lg[:, :N],
                        wg_sb[:, kd, :],
                        xc[:, kd, :N],
                        start=(kd == 0),
                        stop=(kd == KD - 1),
                    )
                ex = epool.tile([P, CHUNK], bf16, tag="ex")
                nc.vector.memset(ex[:, :N], 0.0)
                nc.scalar.activation(ex[:E, :N], lg[:, :N], AF.Exp)
                wd_ps = ps_w.tile([2, CHUNK], f32, tag="wd")
                nc.tensor.matmul(wd_ps[:, :N], sel_sb[:], ex[:, :N], start=True, stop=True)
                wd_sb = epool.tile([2, CHUNK], f32, tag="wd_sb")
                nc.scalar.copy(wd_sb[:, :N], wd_ps[:, :N])
                nc.gpsimd.dma_start(wd_dram[:, c0 : c0 + N], wd_sb[:, :N])

                # Broadcast [1, N] -> [128, N] via a step-0 DRAM read, then
                # w = numer * (1 / denom).
                den_b = wpool2.tile([P, CHUNK], f32, tag="den")
                nc.gpsimd.dma_start(
                    den_b[:, :N], wd_dram[0:1, c0 : c0 + N].to_broadcast([P, N])
                )
                num_b = wpool2.tile([P, CHUNK], f32, tag="num")
                nc.gpsimd.dma_start(
                    num_b[:, :N], wd_dram[1:2, c0 : c0 + N].to_broadcast([P, N])
                )
                wch = wpool2.tile([P, CHUNK], f32, tag="wch")
                nc.vector.reciprocal(wch[:, :N], den_b[:, :N])
                nc.vector.tensor_mul(wch[:, :N], wch[:, :N], num_b[:, :N])

                # GEMM1: h^T = relu(W1^T @ x^T + b1), evicted to SBUF as bf16.
                hT = hpool.tile([P, MH, CHUNK], bf16, tag="h")
                for mh in range(MH):
                    ph = ps_h.tile([P, CHUNK], f32, tag="ph")
                    for kd in range(KD):
                        nc.tensor.matmul(
                            ph[:, :N],
                            w1_sb[:, kd, mh * P : (mh + 1) * P],
                            xc[:, kd, :N],
                            start=(kd == 0),
                            stop=(kd == KD - 1),
                        )
                    nc.scalar.activation(
                        hT[:, mh, :N], ph[:, :N], AF.Relu, bias=b1_sb[:, mh : mh + 1]
                    )

                # GEMM2: y^T = W2^T @ h^T, evicted as (y + b2) * w.
                for mo in range(MO):
                    py = ps_y.tile([P, CHUNK], f32, tag="py")
                    for kh in range(MH):
                        nc.tensor.matmul(
                            py[:, :N],
                            w2_sb[:, kh, mo * P : (mo + 1) * P],
                            hT[:, kh, :N],
                            start=(kh == 0),
                            stop=(kh == MH - 1),
                        )
                    ob = outpool.tile([P, CHUNK], f32, tag="ob")
                    nc.vector.scalar_tensor_tensor(
                        ob[:, :N],
                        py[:, :N],
                        b2_sb[:, mo : mo + 1],
                        wch[:, :N],
                        op0=ALU.add,
                        op1=ALU.mult,
                    )
                    nc.sync.dma_start(yT[mo * P : (mo + 1) * P, c0 : c0 + N], ob[:, :N])
                c0 += N
    return _normalize_sync_waits(nc)


def kernel(**inputs):
    x = np.ascontiguousarray(np.asarray(inputs["x"], dtype=np.float32))
    Wg = np.ascontiguousarray(np.asarray(inputs["Wg"], dtype=np.float32))
    W1 = np.asarray(inputs["W1"], dtype=np.float32)
    b1 = np.asarray(inputs["b1"], dtype=np.float32)
    W2 = np.asarray(inputs["W2"], dtype=np.float32)
    b2 = np.asarray(inputs["b2"], dtype=np.float32)
    k = int(np.asarray(inputs["k"]))

    B, D = x.shape
    E = Wg.shape[1]
    H = W1.shape[2]
    O = W2.shape[2]
    assert E == N_CORES, f"expert-per-core layout expects E == 8, got {E}"

    # Host-side dispatch: pick each token's top-k experts (softmax is
    # monotonic, so top-k on logits == top-k on probs).
    logits = x @ Wg
    kth = np.partition(logits, E - k, axis=1)[:, E - k]  # k-th largest per token
    routed = logits >= kth[:, None]  # [B, E] membership mask
    idx_per_e = [np.nonzero(routed[:, e])[0] for e in range(E)]
    counts = [len(ix) for ix in idx_per_e]

    # Capacity: pad the largest expert's token count to a multiple of 64.
    # Split into <=512-token chunks; keep every chunk >=256 (below that the
    # fixed per-matmul issue/LDWEIGHTS cost stops amortizing) by borrowing
    # from the previous full chunk.
    C = max(CHUNK, -(-max(counts) // 8) * 8)
    chunks = [CHUNK] * (C // CHUNK)
    rem = C % CHUNK
    if rem:
        if rem < 256 and chunks:
            chunks[-1] -= 256 - rem
            rem = 256
        chunks.append(rem)

    nc = _build_program(D, H, O, E, C, chunks)

    in_maps = []
    wg_bf = np.ascontiguousarray(Wg.astype(ml_dtypes.bfloat16))
    for e in range(E):
        idx = idx_per_e[e]
        pad = np.zeros(C, dtype=np.int64)
        pad[: counts[e]] = idx
        xT_e = np.ascontiguousarray(x[pad].T.astype(ml_dtypes.bfloat16))
        sel128 = np.zeros((P, 2), dtype=ml_dtypes.bfloat16)
        sel128[:E, 0] = 1.0
        sel128[e, 1] = 1.0
        in_maps.append(
            {
                "xT": xT_e,
                "w1": np.ascontiguousarray(W1[e].astype(ml_dtypes.bfloat16)),
                "w2": np.ascontiguousarray(W2[e].astype(ml_dtypes.bfloat16)),
                "wg": wg_bf,
                "b1p": np.ascontiguousarray(b1[e].reshape(H // P, P).T),
                "b2p": np.ascontiguousarray(b2[e].reshape(O // P, P).T),
                "sel": sel128,
            }
        )

    res = run_bass_kernel_spmd(nc, in_maps, core_ids=list(range(N_CORES)))
    globals()["_last_results"] = res

    out = np.zeros((B, O), dtype=np.float32)
    for e in range(E):
        cnt = counts[e]
        if cnt:
            yT_e = res.results[e]["yT"]
            out[idx_per_e[e]] += yT_e[:, :cnt].T
    return out


# revision 27
# speedup vs baseline: 1.0025x; 1.0004x over previous
"""MoE layer (router + top-k dispatch + per-expert FFN + weighted combine)
on 8 Trainium2 NeuronCores.

Sharding strategy (expert-parallel, host-side dispatch):
  - Core e owns expert e's weights (W1[e], W2[e], b1[e], b2[e]).
  - The host computes the top-k routing to decide WHICH tokens go to which
    core (the dispatch step of the sharding), gathers each expert's tokens,
    and ships them transposed ([D, C] token-minor) so both FFN GEMMs run
    with contraction on the partition axis and zero on-device transposes.
  - Each core re-computes the router (x @ Wg -> softmax) on-device for its
    own tokens to obtain the combine weight probs[token, e]; for a token
    routed to expert e that probability IS the reference combine weight.
  - Device output is w * (relu(x @ W1 + b1) @ W2 + b2), transposed [O, C].
  - The host unshard step scatters-adds each expert's token columns back
    into the [B, O] output (token indices are unique within one expert).

Compute is bf16 (fp32 PSUM accumulation); combine weights stay fp32.
"""

import numpy as np
import ml_dtypes
import bass_rust

import concourse.bass as bass
import concourse.mybir as mybir
import concourse.tile as tile
from concourse.bass_utils import run_bass_kernel_spmd

P = 128
N_CORES = 8
CHUNK = 512

def _normalize_sync_waits(nc):
    """The walrus build in this toolchain rejects >1 sync wait on a single
    instruction (setupSyncWait: "Too many sync wait commands"), while Tile's
    semaphore assignment freely emits several. Hoist all but one wait of each
    instruction onto same-engine NOPs placed immediately before it — the
    engine stream is in-order, so stalling at the NOPs is semantically
    identical to a multi-wait instruction."""
    count = 0
    for f in nc.m.functions:
        for bb in f.blocks:
            out = []
            changed = False
            for ins in bb.instructions:
                si = ins.sync_info
                if si is not None and len(si.on_wait) > 1:
                    waits = list(si.on_wait)
                    for w in waits[:-1]:
                        count += 1
                        out.append(
                            mybir.InstNoOp(
                                name=f"I-nw{count}",
                                ins=[],
                                outs=[],
                                engine=ins.engine,
                                sync_info=bass_rust.SyncInfo(
                                    on_wait=[w], on_update=[]
                                ),
                            )
                        )
                    ins.sync_info = bass_rust.SyncInfo(
                        on_wait=[waits[-1]], on_update=list(si.on_update)
                    )
                    changed = True
                out.append(ins)
            if changed:
                bb.instructions = out
    return nc


def _build_program(D, H, O, E, C, chunks):
    f32, bf16 = mybir.dt.float32, mybir.dt.bfloat16
    KD, MH, MO = D // P, H // P, O // P
    AF = mybir.ActivationFunctionType
    ALU = mybir.AluOpType

    nc = bass.Bass()
    xT = nc.declare_dram_parameter("xT", [D, C], bf16, isOutput=False)
    w1 = nc.declare_dram_parameter("w1", [D, H], bf16, isOutput=False)
    w2 = nc.declare_dram_parameter("w2", [H, O], bf16, isOutput=False)
    wg = nc.declare_dram_parameter("wg", [D, E], bf16, isOutput=False)
    b1p = nc.declare_dram_parameter("b1p", [P, MH], f32, isOutput=False)
    b2p = nc.declare_dram_parameter("b2p", [P, MO], f32, isOutput=False)
    sel = nc.declare_dram_parameter("sel", [P, 2], bf16, isOutput=False)
    yT = nc.declare_dram_parameter("yT", [O, C], f32, isOutput=True)

    with tile.TileContext(nc) as tc:
        with (
            tc.tile_pool(name="weights", bufs=1) as wpool,
            tc.tile_pool(name="dram", bufs=1, space="DRAM") as dram,
            tc.tile_pool(name="xc", bufs=2) as xcpool,
            tc.tile_pool(name="h", bufs=1) as hpool,
            tc.tile_pool(name="ex", bufs=2) as epool,
            tc.tile_pool(name="wch", bufs=2) as wpool2,
            tc.tile_pool(name="ob", bufs=4) as outpool,
            tc.tile_pool(name="ps_h", bufs=4, space="PSUM") as ps_h,
            tc.tile_pool(name="ps_y", bufs=2, space="PSUM") as ps_y,
            tc.tile_pool(name="ps_r", bufs=1, space="PSUM") as ps_r,
            tc.tile_pool(name="ps_w", bufs=1, space="PSUM") as ps_w,
        ):
            # Small latency-critical inputs first (program order sets DMA
            # priority): the router needs only wg + the first token chunk, so
            # PE work starts ~11us in instead of behind 16MB of weights. DMA
            # *trigger* issue costs ~0.6-1us of queue time each, so keep the
            # count low: 4 H-blocks for w1 (GEMM1 group mh only reads one
            # 128-col slice per kd tile, so the first 2MB block unblocks it),
            # 2 halves for w2 (not needed until GEMM2, ~60us in).
            wg_sb = wpool.tile([P, KD, E], bf16)
            b1_sb = wpool.tile([P, MH], f32)
            nc.gpsimd.dma_start(b1_sb[:], b1p[:])
            b2_sb = wpool.tile([P, MO], f32)
            nc.gpsimd.dma_start(b2_sb[:], b2p[:])
            sel_sb = wpool.tile([P, 2], bf16)
            nc.gpsimd.dma_start(sel_sb[:], sel[:])

            xT_r = xT.rearrange("(kd p) c -> p kd c", p=P)
            xc0 = xcpool.tile([P, KD, CHUNK], bf16, tag="xc")
            nc.sync.dma_start(xc0[:, :, : chunks[0]], xT_r[:, :, : chunks[0]])
            nc.sync.dma_start(wg_sb[:], wg.rearrange("(kd p) e -> p kd e", p=P))

            # First H-block as 8 small 2D DMAs: 2D triggers issue in ~0.7us
            # vs ~9us of descriptor generation for the merged 3D form, so
            # GEMM1's first groups unblock sooner. Later blocks merged — their
            # trigger time hides behind chunk-0 compute.
            w1_sb = wpool.tile([P, KD, H], bf16)
            w1_r = w1.rearrange("(kd p) h -> p kd h", p=P)
            HB = H // 4
            for kd in range(KD):
                nc.sync.dma_start(w1_sb[:, kd, :HB], w1_r[:, kd, :HB])
            for hb in range(1, 4):
                nc.sync.dma_start(
                    w1_sb[:, :, hb * HB : (hb + 1) * HB],
                    w1_r[:, :, hb * HB : (hb + 1) * HB],
                )
            w2_sb = wpool.tile([P, MH, O], bf16)
            w2_r = w2.rearrange("(kh p) o -> p kh o", p=P)
            for j in range(0, MH, MH // 2):
                nc.sync.dma_start(
                    w2_sb[:, j : j + MH // 2, :], w2_r[:, j : j + MH // 2, :]
                )

            wd_dram = dram.tile([2, C], f32)  # row 0: softmax denom, row 1: exp[e]

            c0 = 0
            for ci, N in enumerate(chunks):
                if ci == 0:
                    xc = xc0
                else:
                    xc = xcpool.tile([P, KD, CHUNK], bf16, tag="xc")
                    nc.gpsimd.dma_start(xc[:, :, :N], xT_r[:, :, c0 : c0 + N])

                # Router: logits^T = Wg^T @ x^T, exp, then one matmul against
                # [ones | onehot(e)] gives [denom; numer] per token.
                lg = ps_r.tile([E, CHUNK], f32, tag="lg")
                for kd in range(KD):
                    nc.tensor.matmul(
                        lg[:, :N],
                        wg_sb[:, kd, :],
                        xc[:, kd, :N],
                        start=(kd == 0),
                        stop=(kd == KD - 1),
                    )
                ex = epool.tile([P, CHUNK], bf16, tag="ex")
                nc.vector.memset(ex[:, :N], 0.0)
                nc.scalar.activation(ex[:E, :N], lg[:, :N], AF.Exp)
                wd_ps = ps_w.tile([2, CHUNK], f32, tag="wd")
                nc.tensor.matmul(wd_ps[:, :N], sel_sb[:], ex[:, :N], start=True, stop=True)
                wd_sb = epool.tile([2, CHUNK], f32, tag="wd_sb")
                nc.scalar.copy(wd_sb[:, :N], wd_ps[:, :N])
                nc.gpsimd.dma_start(wd_dram[:, c0 : c0 + N], wd_sb[:, :N])

                # Broadcast [1, N] -> [128, N] via a step-0 DRAM read, then
                # w = numer * (1 / denom).
                den_b = wpool2.tile([P, CHUNK], f32, tag="den")
                nc.gpsimd.dma_start(
                    den_b[:, :N], wd_dram[0:1, c0 : c0 + N].to_broadcast([P, N])
                )
                num_b = wpool2.tile([P, CHUNK], f32, tag="num")
                nc.gpsimd.dma_start(
                    num_b[:, :N], wd_dram[1:2, c0 : c0 + N].to_broadcast([P, N])
                )
                wch = wpool2.tile([P, CHUNK], f32, tag="wch")
                nc.vector.reciprocal(wch[:, :N], den_b[:, :N])
                nc.vector.tensor_mul(wch[:, :N], wch[:, :N], num_b[:, :N])

                # GEMM1: h^T = relu(W1^T @ x^T + b1), evicted to SBUF as bf16.
                hT = hpool.tile([P, MH, CHUNK], bf16, tag="h")
                for mh in range(MH):
                    ph = ps_h.tile([P, CHUNK], f32, tag="ph")
                    for kd in range(KD):
                        nc.tensor.matmul(
                            ph[:, :N],
                            w1_sb[:, kd, mh * P : (mh + 1) * P],
                            xc[:, kd, :N],
                            start=(kd == 0),
                            stop=(kd == KD - 1),
                        )
                    nc.scalar.activation(
                        hT[:, mh, :N], ph[:, :N], AF.Relu, bias=b1_sb[:, mh : mh + 1]
                    )

                # GEMM2: y^T = W2^T @ h^T, evicted as (y + b2) * w.
                for mo in range(MO):
                    py = ps_y.tile([P, CHUNK], f32, tag="py")
                    for kh in range(MH):
                        nc.tensor.matmul(
                            py[:, :N],
                            w2_sb[:, kh, mo * P : (mo + 1) * P],
                            hT[:, kh, :N],
                            start=(kh == 0),
                            stop=(kh == MH - 1),
                        )
                    ob = outpool.tile([P, CHUNK], f32, tag="ob")
                    nc.vector.scalar_tensor_tensor(
                        ob[:, :N],
                        py[:, :N],
                        b2_sb[:, mo : mo + 1],
                        wch[:, :N],
                        op0=ALU.add,
                        op1=ALU.mult,
                    )
                    nc.sync.dma_start(yT[mo * P : (mo + 1) * P, c0 : c0 + N], ob[:, :N])
                c0 += N
    return _normalize_sync_waits(nc)


def kernel(**inputs):
    x = np.ascontiguousarray(np.asarray(inputs["x"], dtype=np.float32))
    Wg = np.ascontiguousarray(np.asarray(inputs["Wg"], dtype=np.float32))
    W1 = np.asarray(inputs["W1"], dtype=np.float32)
    b1 = np.asarray(inputs["b1"], dtype=np.float32)
    W2 = np.asarray(inputs["W2"], dtype=np.float32)
    b2 = np.asarray(inputs["b2"], dtype=np.float32)
    k = int(np.asarray(inputs["k"]))

    B, D = x.shape
    E = Wg.shape[1]
    H = W1.shape[2]
    O = W2.shape[2]
    assert E == N_CORES, f"expert-per-core layout expects E == 8, got {E}"

    # Host-side dispatch: pick each token's top-k experts (softmax is
    # monotonic, so top-k on logits == top-k on probs).
    logits = x @ Wg
    kth = np.partition(logits, E - k, axis=1)[:, E - k]  # k-th largest per token
    routed = logits >= kth[:, None]  # [B, E] membership mask
    idx_per_e = [np.nonzero(routed[:, e])[0] for e in range(E)]
    counts = [len(ix) for ix in idx_per_e]

    # Capacity: pad the largest expert's token count to a multiple of 64.
    # Split into <=512-token chunks; keep every chunk >=256 (below that the
    # fixed per-matmul issue/LDWEIGHTS cost stops amortizing) by borrowing
    # from the previous full chunk.
    C = max(CHUNK, -(-max(counts) // 8) * 8)
    chunks = [CHUNK] * (C // CHUNK)
    rem = C % CHUNK
    if rem:
        if rem < 256 and chunks:
            chunks[-1] -= 256 - rem
            rem = 256
        chunks.append(rem)

    nc = _build_program(D, H, O, E, C, chunks)

    in_maps = []
    wg_bf = np.ascontiguousarray(Wg.astype(ml_dtypes.bfloat16))
    for e in range(E):
        idx = idx_per_e[e]
        pad = np.zeros(C, dtype=np.int64)
        pad[: counts[e]] = idx
        xT_e = np.ascontiguousarray(x[pad].T.astype(ml_dtypes.bfloat16))
        sel128 = np.zeros((P, 2), dtype=ml_dtypes.bfloat16)
        sel128[:E, 0] = 1.0
        sel128[e, 1] = 1.0
        in_maps.append(
            {
                "xT": xT_e,
                "w1": np.ascontiguousarray(W1[e].astype(ml_dtypes.bfloat16)),
                "w2": np.ascontiguousarray(W2[e].astype(ml_dtypes.bfloat16)),
                "wg": wg_bf,
                "b1p": np.ascontiguousarray(b1[e].reshape(H // P, P).T),
                "b2p": np.ascontiguousarray(b2[e].reshape(O // P, P).T),
                "sel": sel128,
            }
        )

    res = run_bass_kernel_spmd(nc, in_maps, core_ids=list(range(N_CORES)))
    globals()["_last_results"] = res

    out = np.zeros((B, O), dtype=np.float32)
    for e in range(E):
        cnt = counts[e]
        if cnt:
            yT_e = res.results[e]["yT"]
            out[idx_per_e[e]] += yT_e[:, :cnt].T
    return out


# revision 29
# speedup vs baseline: 1.0051x; 1.0026x over previous
"""MoE layer (router + top-k dispatch + per-expert FFN + weighted combine)
on 8 Trainium2 NeuronCores.

Sharding strategy (expert-parallel, host-side dispatch):
  - Core e owns expert e's weights (W1[e], W2[e], b1[e], b2[e]).
  - The host computes the top-k routing to decide WHICH tokens go to which
    core (the dispatch step of the sharding), gathers each expert's tokens,
    and ships them transposed ([D, C] token-minor) so both FFN GEMMs run
    with contraction on the partition axis and zero on-device transposes.
  - Each core re-computes the router (x @ Wg -> softmax) on-device for its
    own tokens to obtain the combine weight probs[token, e]; for a token
    routed to expert e that probability IS the reference combine weight.
  - Device output is w * (relu(x @ W1 + b1) @ W2 + b2), transposed [O, C].
  - The host unshard step scatters-adds each expert's token columns back
    into the [B, O] output (token indices are unique within one expert).

Compute is bf16 (fp32 PSUM accumulation); combine weights stay fp32.
"""

import numpy as np
import ml_dtypes
import bass_rust

import concourse.bass as bass
import concourse.mybir as mybir
import concourse.tile as tile
from concourse.bass_utils import run_bass_kernel_spmd

P = 128
N_CORES = 8
CHUNK = 512

def _normalize_sync_waits(nc):
    """The walrus build in this toolchain rejects >1 sync wait on a single
    instruction (setupSyncWait: "Too many sync wait commands"), while Tile's
    semaphore assignment freely emits several. Hoist all but one wait of each
    instruction onto same-engine NOPs placed immediately before it — the
    engine stream is in-order, so stalling at the NOPs is semantically
    identical to a multi-wait instruction."""
    count = 0
    for f in nc.m.functions:
        for bb in f.blocks:
            out = []
            changed = False
            for ins in bb.instructions:
                si = ins.sync_info
                if si is not None and len(si.on_wait) > 1:
                    waits = list(si.on_wait)
                    for w in waits[:-1]:
                        count += 1
                        out.append(
                            mybir.InstNoOp(
                                name=f"I-nw{count}",
                                ins=[],
                                outs=[],
                                engine=ins.engine,
                                sync_info=bass_rust.SyncInfo(
                                    on_wait=[w], on_update=[]
                                ),
                            )
                        )
                    ins.sync_info = bass_rust.SyncInfo(
                        on_wait=[waits[-1]], on_update=list(si.on_update)
                    )
                    changed = True
                out.append(ins)
            if changed:
                bb.instructions = out
    return nc


def _build_program(D, H, O, E, C, chunks):
    f32, bf16 = mybir.dt.float32, mybir.dt.bfloat16
    KD, MH, MO = D // P, H // P, O // P
    AF = mybir.ActivationFunctionType
    ALU = mybir.AluOpType

    nc = bass.Bass()
    xT = nc.declare_dram_parameter("xT", [D, C], bf16, isOutput=False)
    w1 = nc.declare_dram_parameter("w1", [D, H], bf16, isOutput=False)
    w2 = nc.declare_dram_parameter("w2", [H, O], bf16, isOutput=False)
    wg = nc.declare_dram_parameter("wg", [D, E], bf16, isOutput=False)
    b1p = nc.declare_dram_parameter("b1p", [P, MH], f32, isOutput=False)
    b2p = nc.declare_dram_parameter("b2p", [P, MO], f32, isOutput=False)
    sel = nc.declare_dram_parameter("sel", [P, 2], bf16, isOutput=False)
    yT = nc.declare_dram_parameter("yT", [O, C], f32, isOutput=True)

    with tile.TileContext(nc) as tc:
        with (
            tc.tile_pool(name="weights", bufs=1) as wpool,
            tc.tile_pool(name="dram", bufs=1, space="DRAM") as dram,
            tc.tile_pool(name="xc", bufs=2) as xcpool,
            tc.tile_pool(name="h", bufs=1) as hpool,
            tc.tile_pool(name="ex", bufs=2) as epool,
            tc.tile_pool(name="wch", bufs=2) as wpool2,
            tc.tile_pool(name="ob", bufs=4) as outpool,
            tc.tile_pool(name="ps_h", bufs=4, space="PSUM") as ps_h,
            tc.tile_pool(name="ps_y", bufs=2, space="PSUM") as ps_y,
            tc.tile_pool(name="ps_r", bufs=1, space="PSUM") as ps_r,
            tc.tile_pool(name="ps_w", bufs=1, space="PSUM") as ps_w,
        ):
            # Small latency-critical inputs first (program order sets DMA
            # priority): the router needs only wg + the first token chunk, so
            # PE work starts ~11us in instead of behind 16MB of weights. DMA
            # *trigger* issue costs ~0.6-1us of queue time each, so keep the
            # count low: 4 H-blocks for w1 (GEMM1 group mh only reads one
            # 128-col slice per kd tile, so the first 2MB block unblocks it),
            # 2 halves for w2 (not needed until GEMM2, ~60us in).
            wg_sb = wpool.tile([P, KD, E], bf16)
            b1_sb = wpool.tile([P, MH], f32)
            nc.gpsimd.dma_start(b1_sb[:], b1p[:])
            b2_sb = wpool.tile([P, MO], f32)
            nc.gpsimd.dma_start(b2_sb[:], b2p[:])
            sel_sb = wpool.tile([P, 2], bf16)
            nc.gpsimd.dma_start(sel_sb[:], sel[:])

            xT_r = xT.rearrange("(kd p) c -> p kd c", p=P)
            xc0 = xcpool.tile([P, KD, CHUNK], bf16, tag="xc")
            nc.sync.dma_start(xc0[:, :, : chunks[0]], xT_r[:, :, : chunks[0]])
            nc.sync.dma_start(wg_sb[:], wg.rearrange("(kd p) e -> p kd e", p=P))

            # First H-block as 8 small 2D DMAs: 2D triggers issue in ~0.7us
            # vs ~9us of descriptor generation for the merged 3D form, so
            # GEMM1's first groups unblock sooner. Later blocks merged — their
            # trigger time hides behind chunk-0 compute.
            w1_sb = wpool.tile([P, KD, H], bf16)
            w1_r = w1.rearrange("(kd p) h -> p kd h", p=P)
            HB = H // 4
            for kd in range(KD):
                nc.sync.dma_start(w1_sb[:, kd, :HB], w1_r[:, kd, :HB])
            for hb in range(1, 4):
                nc.sync.dma_start(
                    w1_sb[:, :, hb * HB : (hb + 1) * HB],
                    w1_r[:, :, hb * HB : (hb + 1) * HB],
                )
            w2_sb = wpool.tile([P, MH, O], bf16)
            w2_r = w2.rearrange("(kh p) o -> p kh o", p=P)
            for j in range(0, MH, MH // 2):
                nc.sync.dma_start(
                    w2_sb[:, j : j + MH // 2, :], w2_r[:, j : j + MH // 2, :]
                )

            wd_dram = dram.tile([2, C], f32)  # row 0: softmax denom, row 1: exp[e]

            c0 = 0
            for ci, N in enumerate(chunks):
                if ci == 0:
                    xc = xc0
                else:
                    xc = xcpool.tile([P, KD, CHUNK], bf16, tag="xc")
                    nc.gpsimd.dma_start(xc[:, :, :N], xT_r[:, :, c0 : c0 + N])

                # Router: logits^T = Wg^T @ x^T, exp, then one matmul against
                # [ones | onehot(e)] gives [denom; numer] per token.
                lg = ps_r.tile([E, CHUNK], f32, tag="lg")
                for kd in range(KD):
                    nc.tensor.matmul(
                        lg[:, :N],
                        wg_sb[:, kd, :],
                        xc[:, kd, :N],
                        start=(kd == 0),
                        stop=(kd == KD - 1),
                    )
                ex = epool.tile([P, CHUNK], bf16, tag="ex")
                nc.vector.memset(ex[:, :N], 0.0)
                nc.scalar.activation(ex[:E, :N], lg[:, :N], AF.Exp)
                wd_ps = ps_w.tile([2, CHUNK], f32, tag="wd")
                nc.tensor.matmul(wd_ps[:, :N], sel_sb[:], ex[:, :N], start=True, stop=True)
                wd_sb = epool.tile([2, CHUNK], f32, tag="wd_sb")
                nc.scalar.copy(wd_sb[:, :N], wd_ps[:, :N])
                nc.gpsimd.dma_start(wd_dram[:, c0 : c0 + N], wd_sb[:, :N])

                # Broadcast [1, N] -> [128, N] via a step-0 DRAM read, then
                # w = numer * (1 / denom).
                den_b = wpool2.tile([P, CHUNK], f32, tag="den")
                nc.gpsimd.dma_start(
                    den_b[:, :N], wd_dram[0:1, c0 : c0 + N].to_broadcast([P, N])
                )
                num_b = wpool2.tile([P, CHUNK], f32, tag="num")
                nc.gpsimd.dma_start(
                    num_b[:, :N], wd_dram[1:2, c0 : c0 + N].to_broadcast([P, N])
                )
                wch = wpool2.tile([P, CHUNK], f32, tag="wch")
                nc.vector.reciprocal(wch[:, :N], den_b[:, :N])
                nc.vector.tensor_mul(wch[:, :N], wch[:, :N], num_b[:, :N])

                # GEMM1: h^T = relu(W1^T @ x^T + b1), evicted to SBUF as bf16.
                # h is split into two half-tiles so the next chunk's GEMM1 can
                # start evicting into the first half as soon as this chunk's
                # GEMM2 has consumed it (tile deps are per-tile, not
                # per-region) — removes the chunk-boundary WAW bubble.
                hT_a = hpool.tile([P, MH // 2, CHUNK], bf16, tag="h_a")
                hT_b = hpool.tile([P, MH // 2, CHUNK], bf16, tag="h_b")

                def h_slice(kh, N=N, hT_a=hT_a, hT_b=hT_b):
                    t = hT_a if kh < MH // 2 else hT_b
                    return t[:, kh % (MH // 2), :N]

                for mh in range(MH):
                    ph = ps_h.tile([P, CHUNK], f32, tag="ph")
                    for kd in range(KD):
                        nc.tensor.matmul(
                            ph[:, :N],
                            w1_sb[:, kd, mh * P : (mh + 1) * P],
                            xc[:, kd, :N],
                            start=(kd == 0),
                            stop=(kd == KD - 1),
                        )
                    nc.scalar.activation(
                        h_slice(mh), ph[:, :N], AF.Relu, bias=b1_sb[:, mh : mh + 1]
                    )

                # GEMM2: y^T = W2^T @ h^T, evicted as (y + b2) * w.
                for mo in range(MO):
                    py = ps_y.tile([P, CHUNK], f32, tag="py")
                    for kh in range(MH):
                        nc.tensor.matmul(
                            py[:, :N],
                            w2_sb[:, kh, mo * P : (mo + 1) * P],
                            h_slice(kh),
                            start=(kh == 0),
                            stop=(kh == MH - 1),
                        )
                    ob = outpool.tile([P, CHUNK], f32, tag="ob")
                    nc.vector.scalar_tensor_tensor(
                        ob[:, :N],
                        py[:, :N],
                        b2_sb[:, mo : mo + 1],
                        wch[:, :N],
                        op0=ALU.add,
                        op1=ALU.mult,
                    )
                    nc.sync.dma_start(yT[mo * P : (mo + 1) * P, c0 : c0 + N], ob[:, :N])
                c0 += N
    return _normalize_sync_waits(nc)


def kernel(**inputs):
    x = np.ascontiguousarray(np.asarray(inputs["x"], dtype=np.float32))
    Wg = np.ascontiguousarray(np.asarray(inputs["Wg"], dtype=np.float32))
    W1 = np.asarray(inputs["W1"], dtype=np.float32)
    b1 = np.asarray(inputs["b1"], dtype=np.float32)
    W2 = np.asarray(inputs["W2"], dtype=np.float32)
    b2 = np.asarray(inputs["b2"], dtype=np.float32)
    k = int(np.asarray(inputs["k"]))

    B, D = x.shape
    E = Wg.shape[1]
    H = W1.shape[2]
    O = W2.shape[2]
    assert E == N_CORES, f"expert-per-core layout expects E == 8, got {E}"

    # Host-side dispatch: pick each token's top-k experts (softmax is
    # monotonic, so top-k on logits == top-k on probs).
    logits = x @ Wg
    kth = np.partition(logits, E - k, axis=1)[:, E - k]  # k-th largest per token
    routed = logits >= kth[:, None]  # [B, E] membership mask
    idx_per_e = [np.nonzero(routed[:, e])[0] for e in range(E)]
    counts = [len(ix) for ix in idx_per_e]

    # Capacity: pad the largest expert's token count to a multiple of 64.
    # Split into <=512-token chunks; keep every chunk >=256 (below that the
    # fixed per-matmul issue/LDWEIGHTS cost stops amortizing) by borrowing
    # from the previous full chunk.
    C = max(CHUNK, -(-max(counts) // 8) * 8)
    chunks = [CHUNK] * (C // CHUNK)
    rem = C % CHUNK
    if rem:
        if rem < 256 and chunks:
            chunks[-1] -= 256 - rem
            rem = 256
        chunks.append(rem)

    nc = _build_program(D, H, O, E, C, chunks)

    in_maps = []
    wg_bf = np.ascontiguousarray(Wg.astype(ml_dtypes.bfloat16))
    for e in range(E):
        idx = idx_per_e[e]
        pad = np.zeros(C, dtype=np.int64)
        pad[: counts[e]] = idx
        xT_e = np.ascontiguousarray(x[pad].T.astype(ml_dtypes.bfloat16))
        sel128 = np.zeros((P, 2), dtype=ml_dtypes.bfloat16)
        sel128[:E, 0] = 1.0
        sel128[e, 1] = 1.0
        in_maps.append(
            {
                "xT": xT_e,
                "w1": np.ascontiguousarray(W1[e].astype(ml_dtypes.bfloat16)),
                "w2": np.ascontiguousarray(W2[e].astype(ml_dtypes.bfloat16)),
                "wg": wg_bf,
                "b1p": np.ascontiguousarray(b1[e].reshape(H // P, P).T),
                "b2p": np.ascontiguousarray(b2[e].reshape(O // P, P).T),
                "sel": sel128,
            }
        )

    res = run_bass_kernel_spmd(nc, in_maps, core_ids=list(range(N_CORES)))
    globals()["_last_results"] = res

    out = np.zeros((B, O), dtype=np.float32)
    for e in range(E):
        cnt = counts[e]
        if cnt:
            yT_e = res.results[e]["yT"]
            out[idx_per_e[e]] += yT_e[:, :cnt].T
    return out
